# revision 58
# baseline (speedup 1.0000x reference)
"""MoE FFN (shared SwiGLU + 8 dense-routed SwiGLU experts) on 8 TRN2 NeuronCores.

Strategy: data-parallel over batch (B=16 -> 2 batches/core). The 10 uniform
512->1024->512 SwiGLU units (2 shared halves + 8 experts) run with per-unit
precision: shared units in bf16, expert units in fp8e4m3 DoubleRow matmuls
(2x PE throughput). Expert quantization errors are independent across the 8
experts and average down under the routing weights, keeping total rel err
~1.8e-2 (<2e-2 gate) while cutting PE time to ~0.6x of the bf16 roofline.

All weights stay resident in SBUF (~170KB/partition incl. x), host-repacked
to partition-contiguous layout so the whole working set loads with ~50 large
DMA descriptors (an earlier revision spent 712us of gpsimd time issuing 960
fine-grained weight DMAs). Loop is token-tile-outer / unit-inner so all 10
units' down-projections accumulate directly in PSUM at a common scale
(shared h pre-scaled by SW*C=32768, exact in floating point); one op per
(d-tile, token-tile) drains PSUM with bias + descale fused, alternating
between the scalar and vector engines. With zero up-biases (this problem's
inputs) the whole up-branch scale + h product is a single fused
scalar_tensor_tensor on the DVE, so each h-tile costs one scalar-engine op
(silu) and one DVE op. Measured: 525us vs the 853us bf16 baseline; PE busy
~500us vs the 491.5us mixed-precision roofline, rel err 1.835e-2.
"""
import sys

if "/opt/trn_rl_repo" not in sys.path:
    sys.path.insert(0, "/opt/trn_rl_repo")

import numpy as np
import ml_dtypes

import concourse.bass as bass  # noqa: F401  (registers engine classes)
import concourse.tile as tile
from concourse import bacc, mybir
from concourse import bass_utils

F32 = mybir.dt.float32
BF16 = mybir.dt.bfloat16
FP8 = mybir.dt.float8e4
Silu = mybir.ActivationFunctionType.Silu
ACT = Silu  # debug harnesses may swap to Sigmoid (CoreSim lacks Silu)
Alu = mybir.AluOpType
DR = mybir.MatmulPerfMode.DoubleRow

B, K, D = 16, 1024, 512
HS, HR, E = 2048, 1024, 8
NCORES = 8
BL = B // NCORES          # batches per core = 2
T = BL * K                # tokens per core = 2048
TT = 512                  # token tile (matmul moving dim)
NTT = T // TT             # 4 token tiles per core
NU = 2 + E                # units: 2 shared halves + 8 experts
HU = 1024                 # hidden width of every unit
NH = HU // 128            # 8 h-tiles per unit
ND = D // 128             # 4 d-tiles
NK = D // 128             # 4 contraction tiles for gate/up
WSZ = NK * HU             # per-matrix elements per partition (4096)

SX = 32.0                 # fp8 x scale
SWQ = 1024.0              # fp8 weight scale
C = 32.0                  # h-domain scale for expert fp8 h
SXW = SX * SWQ            # expert gate/up psum scale
SDC = SWQ * C             # common down psum scale (all units)

# per-unit precision: shared halves bf16, experts fp8
UNIT_FP8 = [False, False] + [True] * E
# experts first: the first unit's weights (1MB fp8 vs 3MB bf16) gate PE start
UORDER = list(range(2, NU)) + [0, 1]

# When every up-projection bias is zero (true for this problem's inputs),
# the up-branch scale and the h product fuse into one DVE op:
#   h = (ups * rwC) * silu(g).
# The general path (ts on vector + tt on gpsimd) stays available for
# nonzero biases; _run picks at call time.


def _build(fused):
    nc = bacc.Bacc("TRN2", target_bir_lowering=False, debug=False,
                   num_devices=NCORES)
    # weights packed host-side to SBUF layout: partition-contiguous, one
    # (unit, matrix) block of WSZ elements per partition per block.
    wsb = nc.dram_tensor("wsb", (128, 2 * 3 * WSZ), BF16, kind="ExternalInput")
    we8 = nc.dram_tensor("we8", (128, E * 3 * WSZ), FP8, kind="ExternalInput")
    xTb = nc.dram_tensor("xTb", (128, NK, T), BF16, kind="ExternalInput")
    xT8 = nc.dram_tensor("xT8", (128, NK, T), FP8, kind="ExternalInput")
    gb = nc.dram_tensor("gb", (128, NU, NH), F32, kind="ExternalInput")
    ub = nc.dram_tensor("ub", (128, NU, NH), F32, kind="ExternalInput")
    rw = nc.dram_tensor("rw", (128, NU, NTT), F32, kind="ExternalInput")
    cv = nc.dram_tensor("cv", (128, ND, NTT), F32, kind="ExternalInput")
    outT = nc.dram_tensor("outT", (D, T), F32, kind="ExternalOutput")

    with tile.TileContext(nc) as tc:
        with (
            tc.tile_pool(name="persist", bufs=1) as persist,
            tc.tile_pool(name="hpool", bufs=2) as hpool,
            tc.tile_pool(name="spool", bufs=2) as spool,
            tc.tile_pool(name="dpool", bufs=2) as dpool,
            tc.tile_pool(name="gups", bufs=2, space="PSUM") as gups,
            tc.tile_pool(name="ops", bufs=1, space="PSUM") as opsp,
        ):
            xb = persist.tile([128, NK, T], BF16)
            x8t = persist.tile([128, NK, T], FP8)
            gbt = persist.tile([128, NU, NH], F32)
            ubt = persist.tile([128, NU, NH], F32)
            rwt = persist.tile([128, NU, NTT], F32)
            cvt = persist.tile([128, ND, NTT], F32)

            # PE pstate warmup: dummy matmuls over a zeroed tile run during
            # the ~4us window between the engine preamble and the first
            # weight DMA landing, so the real stream starts at full clock
            # (measured: first ~20us of matmuls otherwise run at ~2x cycle
            # time). The accumulator is never read.
            warm = persist.tile([128, TT], BF16, name="warm")
            nc.vector.memset(warm[:], 0)
            wps = gups.tile([128, TT], F32, tag="g", name="warmps")
            for i in range(4):
                nc.tensor.matmul(wps[:], warm[:, 0:128], warm[:],
                                 start=(i == 0), stop=(i == 3))

            wtiles = {}
            for idx, u in enumerate(UORDER):
                fp8u = UNIT_FP8[u]
                dt_ = FP8 if fp8u else BF16
                src = we8 if fp8u else wsb
                base = (u - 2 if fp8u else u) * 3 * WSZ
                wgt = persist.tile([128, NK, HU], dt_, name=f"wg{u}")
                wut = persist.tile([128, NK, HU], dt_, name=f"wu{u}")
                wdt = persist.tile([128, NH, D], dt_, name=f"wd{u}")
                wtiles[u] = (wgt, wut, wdt)
                if idx == 0:
                    # x8 token tile 0 gates the very first matmul — split by
                    # k-pair so that matmul only waits on the 128KB it reads.
                    # Tables feed the first silu/stt a few us later. Unit 0's
                    # gate/up stream by column half for an early start.
                    nc.sync.dma_start(x8t[:, 0:2, 0:TT], xT8.ap()[:, 0:2, 0:TT])
                    nc.sync.dma_start(x8t[:, 2:4, 0:TT], xT8.ap()[:, 2:4, 0:TT])
                    nc.sync.dma_start(gbt[:], gb.ap()[:])
                    nc.sync.dma_start(ubt[:], ub.ap()[:])
                    nc.sync.dma_start(rwt[:], rw.ap()[:])
                    nc.sync.dma_start(cvt[:], cv.ap()[:])
                    for half in range(2):
                        for k in range(NK):
                            nc.gpsimd.dma_start(
                                wgt[:, k, half * 512:(half + 1) * 512],
                                src.ap()[:, base + k * HU + half * 512:
                                          base + k * HU + (half + 1) * 512])
                        for k in range(NK):
                            nc.sync.dma_start(
                                wut[:, k, half * 512:(half + 1) * 512],
                                src.ap()[:, base + WSZ + k * HU + half * 512:
                                          base + WSZ + k * HU + (half + 1) * 512])
                    nc.gpsimd.dma_start(wdt[:],
                                        src.ap()[:, base + 2 * WSZ:base + 3 * WSZ])
                    nc.sync.dma_start(xb[:, :, 0:TT], xTb.ap()[:, :, 0:TT])
                    nc.sync.dma_start(x8t[:, :, TT:], xT8.ap()[:, :, TT:])
                    nc.sync.dma_start(xb[:, :, TT:], xTb.ap()[:, :, TT:])
                else:
                    # experts stream just-in-time on gpsimd (~1.5MB per
                    # 10.2us PE window); the late-needed shared weights ride
                    # the sync queue behind x. (Routing any weights via the
                    # scalar-engine DMA queue measured consistently ~6us
                    # slower — its transfers arbitrate poorly here.)
                    q = nc.gpsimd if fp8u else nc.sync
                    for wt, off in [(wgt, 0), (wut, WSZ), (wdt, 2 * WSZ)]:
                        q.dma_start(wt[:],
                                    src.ap()[:, base + off:base + off + WSZ])

            # The PE stream is software-pipelined by one unit: unit (t,ui)'s
            # down matmuls are emitted after unit (t,ui+1)'s gate/up, so the
            # silu->stt h-chain always has a full unit window (10-20us) of
            # PE cover instead of ~6us — this removes the pipeline-fill
            # stalls (and their pstate drops) in the first ~30us and hides
            # the t-boundary drains. Requires hpool bufs=2.
            odsts = {}

            def emit_down(t, ui, fp8u, wdt, hts):
                if ui == 0:
                    odsts[t] = [opsp.tile([128, TT], F32, tag=f"o{di}",
                                          name=f"o{di}_t{t}")
                                for di in range(ND)]
                odst = odsts[t]
                last = ui == NU - 1
                # on the very last unit of the kernel, close each d-tile's
                # accumulation group early (di-major) so the final drains
                # overlap the remaining down matmuls
                dimaj = last and t == NTT - 1
                if fp8u:
                    kds = ([(kp, di) for di in range(ND)
                            for kp in range(NH // 2)] if dimaj else
                           [(kp, di) for kp in range(NH // 2)
                            for di in range(ND)])
                    for kp, di in kds:
                        nc.tensor.matmul(
                            odst[di][:],
                            wdt[:, 2 * kp:2 * kp + 2,
                                di * 128:(di + 1) * 128],
                            hts[:, 2 * kp:2 * kp + 2, :],
                            start=(ui == 0 and kp == 0),
                            stop=(last and kp == NH // 2 - 1),
                            perf_mode=DR, skip_group_check=True)
                else:
                    kds = ([(k, di) for di in range(ND)
                            for k in range(NH)] if dimaj else
                           [(k, di) for k in range(NH)
                            for di in range(ND)])
                    for k, di in kds:
                        nc.tensor.matmul(
                            odst[di][:],
                            wdt[:, k, di * 128:(di + 1) * 128],
                            hts[:, k, :],
                            start=(ui == 0 and k == 0),
                            stop=(last and k == NH - 1),
                            skip_group_check=True)
                if last:
                    tok = slice(t * TT, (t + 1) * TT)
                    for di in range(ND):
                        dtl = dpool.tile([128, TT], F32, tag="d")
                        if di % 2 == 0:
                            nc.scalar.activation(
                                dtl[:], odst[di][:],
                                mybir.ActivationFunctionType.Identity,
                                bias=cvt[:, di, t:t + 1], scale=1.0 / SDC)
                        else:
                            nc.vector.tensor_scalar(dtl[:], odst[di][:],
                                                    1.0 / SDC,
                                                    cvt[:, di, t:t + 1],
                                                    Alu.mult, Alu.add)
                        nc.sync.dma_start(
                            outT.ap()[di * 128:(di + 1) * 128, tok], dtl[:])

            pending = None
            for t in range(NTT):
                tok = slice(t * TT, (t + 1) * TT)
                for ui, u in enumerate(UORDER):
                    fp8u = UNIT_FP8[u]
                    wgt, wut, wdt = wtiles[u]
                    sdt = BF16 if fp8u else F32
                    hts = hpool.tile([128, NH, TT], FP8 if fp8u else BF16,
                                     tag="h8" if fp8u else "hb",
                                     name=f"h_u{u}t{t}")
                    for hi in range(NH):
                        hc = slice(hi * 128, (hi + 1) * 128)
                        gps = gups.tile([128, TT], F32, tag="g")
                        if fp8u:
                            for kp in range(NK // 2):
                                nc.tensor.matmul(
                                    gps[:], wgt[:, 2 * kp:2 * kp + 2, hc],
                                    x8t[:, 2 * kp:2 * kp + 2, tok],
                                    start=(kp == 0), stop=(kp == NK // 2 - 1),
                                    perf_mode=DR)
                        else:
                            for k in range(NK):
                                nc.tensor.matmul(
                                    gps[:], wgt[:, k, hc], xb[:, k, tok],
                                    start=(k == 0), stop=(k == NK - 1))
                        ups = gups.tile([128, TT], F32, tag="u")
                        if fp8u:
                            for kp in range(NK // 2):
                                nc.tensor.matmul(
                                    ups[:], wut[:, 2 * kp:2 * kp + 2, hc],
                                    x8t[:, 2 * kp:2 * kp + 2, tok],
                                    start=(kp == 0), stop=(kp == NK // 2 - 1),
                                    perf_mode=DR)
                        else:
                            for k in range(NK):
                                nc.tensor.matmul(
                                    ups[:], wut[:, k, hc], xb[:, k, tok],
                                    start=(k == 0), stop=(k == NK - 1))
                        sg = spool.tile([128, TT], sdt, tag="sg8" if fp8u else "sgb")
                        nc.scalar.activation(sg[:], gps[:], ACT,
                                             bias=gbt[:, u, hi:hi + 1],
                                             scale=(1.0 / SXW) if fp8u else 1.0)
                        if fused:
                            nc.vector.scalar_tensor_tensor(
                                hts[:, hi, :], ups[:], rwt[:, u, t:t + 1],
                                sg[:], Alu.mult, Alu.mult)
                        else:
                            su = spool.tile([128, TT], sdt,
                                            tag="su8" if fp8u else "sub")
                            nc.vector.tensor_scalar(su[:], ups[:],
                                                    ubt[:, u, hi:hi + 1],
                                                    rwt[:, u, t:t + 1],
                                                    Alu.add, Alu.mult)
                            nc.gpsimd.tensor_tensor(hts[:, hi, :], sg[:],
                                                    su[:], Alu.mult)

                    if pending is not None:
                        emit_down(*pending)
                    pending = (t, ui, fp8u, wdt, hts)
            emit_down(*pending)
    nc.compile()
    return nc


_NC = {}


def _get_nc(fused):
    if fused not in _NC:
        _NC[fused] = _build(fused)
    return _NC[fused]


def _bf16(a):
    return np.ascontiguousarray(np.asarray(a, np.float32)).astype(ml_dtypes.bfloat16)


def _fp8(a, scale):
    return np.ascontiguousarray(
        np.asarray(a, np.float32) * scale).astype(ml_dtypes.float8_e4m3)


def _colmaj(v):
    return np.asarray(v, np.float32).reshape(-1, 128).T


def _sbufpack(w):
    """[D_in, D_out] -> [128, (D_in/128)*D_out] partition-contiguous."""
    din, dout = w.shape
    return w.reshape(din // 128, 128, dout).transpose(1, 0, 2).reshape(128, -1)


def _pack_shared(Ws_gate, bs_gate, Ws_up, bs_up, Ws_down, bs_down,
                 Wr_gate, br_gate, Wr_up, br_up, Wr_down, br_down):
    ws = np.empty((128, 2 * 3 * WSZ), np.float32)
    we = np.empty((128, E * 3 * WSZ), np.float32)
    gbt = np.empty((128, NU, NH), np.float32)
    ubt = np.empty((128, NU, NH), np.float32)
    for u in range(2):
        h0 = slice(u * HU, (u + 1) * HU)
        base = u * 3 * WSZ
        ws[:, base:base + WSZ] = _sbufpack(np.asarray(Ws_gate, np.float32)[:, h0])
        ws[:, base + WSZ:base + 2 * WSZ] = _sbufpack(
            np.asarray(Ws_up, np.float32)[:, h0])
        ws[:, base + 2 * WSZ:base + 3 * WSZ] = _sbufpack(
            np.asarray(Ws_down, np.float32)[h0, :])
        gbt[:, u, :] = _colmaj(bs_gate[h0])
        ubt[:, u, :] = _colmaj(bs_up[h0])
    for e in range(E):
        base = e * 3 * WSZ
        we[:, base:base + WSZ] = _sbufpack(np.asarray(Wr_gate, np.float32)[e])
        we[:, base + WSZ:base + 2 * WSZ] = _sbufpack(
            np.asarray(Wr_up, np.float32)[e])
        we[:, base + 2 * WSZ:base + 3 * WSZ] = _sbufpack(
            np.asarray(Wr_down, np.float32)[e])
        gbt[:, 2 + e, :] = _colmaj(br_gate[e])
        ubt[:, 2 + e, :] = _colmaj(br_up[e]) * SXW
    return _bf16(ws), _fp8(we, SWQ), gbt, ubt


def _run(inputs, trace=False):
    x = np.asarray(inputs["x"], np.float32)
    rweights = np.asarray(inputs["routing_weights"], np.float32)
    wsb, we8, gbt, ubt = _pack_shared(
        np.asarray(inputs["Ws_gate"], np.float32), inputs["bs_gate"],
        np.asarray(inputs["Ws_up"], np.float32), inputs["bs_up"],
        np.asarray(inputs["Ws_down"], np.float32), inputs["bs_down"],
        np.asarray(inputs["Wr_gate"], np.float32), inputs["br_gate"],
        np.asarray(inputs["Wr_up"], np.float32), inputs["br_up"],
        np.asarray(inputs["Wr_down"], np.float32), inputs["br_down"])
    bs_down = np.asarray(inputs["bs_down"], np.float32)
    br_down = np.asarray(inputs["br_down"], np.float32)
    # down-bias vector per batch: bs_down + sum_e rw[b,e]*br_down[e]
    cfull = bs_down[None, :] + rweights @ br_down       # [B, D]

    in_maps = []
    for i in range(NCORES):
        xT = x[BL * i:BL * (i + 1)].reshape(T, D).T     # [D, T]
        # pack x to [128, NK, T]: partition p, block k, token t = xT[k*128+p, t]
        xP = xT.reshape(NK, 128, T).transpose(1, 0, 2)
        rwtab = np.empty((128, NU, NTT), np.float32)
        rwtab[:, :2, :] = SDC
        cvtab = np.empty((128, ND, NTT), np.float32)
        for t in range(NTT):
            bg = BL * i + t // (K // TT)
            for e in range(E):
                rwtab[:, 2 + e, t] = rweights[bg, e] * (C / SXW)
            cvtab[:, :, t] = cfull[bg].reshape(ND, 128).T
        in_maps.append({"xTb": _bf16(xP), "xT8": _fp8(xP, SX),
                        "wsb": wsb, "we8": we8,
                        "gb": gbt, "ub": ubt,
                        "rw": np.ascontiguousarray(rwtab),
                        "cv": np.ascontiguousarray(cvtab)})

    fused = (not np.any(np.asarray(inputs["bs_up"], np.float32))
             and not np.any(np.asarray(inputs["br_up"], np.float32)))
    res = bass_utils.run_bass_kernel_spmd(_get_nc(fused), in_maps,
                                          core_ids=list(range(NCORES)),
                                          trace=trace)
    out = np.empty((B, K, D), np.float32)
    for i in range(NCORES):
        out[BL * i:BL * (i + 1)] = res.results[i]["outT"].T.reshape(BL, K, D)
    return out, res


def kernel(**inputs) -> np.ndarray:
    out, _ = _run(inputs, trace=False)
    return out


# revision 60
# speedup vs baseline: 1.1861x; 1.1861x over previous
"""MoE FFN (shared SwiGLU + 8 dense-routed SwiGLU experts) on 8 TRN2 NeuronCores.

Strategy: data-parallel over batch (B=16 -> 2 batches/core). The 10 uniform
512->1024->512 SwiGLU units (2 shared halves + 8 experts) run with per-unit
precision: shared units in bf16, expert units in fp8e4m3 DoubleRow matmuls
(2x PE throughput). Expert quantization errors are independent across the 8
experts and average down under the routing weights, keeping total rel err
~1.8e-2 (<2e-2 gate) while cutting PE time to ~0.6x of the bf16 roofline.

All weights stay resident in SBUF (~170KB/partition incl. x), host-repacked
to partition-contiguous layout so the whole working set loads with ~50 large
DMA descriptors (an earlier revision spent 712us of gpsimd time issuing 960
fine-grained weight DMAs). Loop is token-tile-outer / unit-inner so all 10
units' down-projections accumulate directly in PSUM at a common scale
(shared h pre-scaled by SW*C=32768, exact in floating point); one op per
(d-tile, token-tile) drains PSUM with bias + descale fused, alternating
between the scalar and vector engines. With zero up-biases (this problem's
inputs) the whole up-branch scale + h product is a single fused
scalar_tensor_tensor on the DVE, so each h-tile costs one scalar-engine op
(silu) and one DVE op. Measured: 525us vs the 853us bf16 baseline; PE busy
~500us vs the 491.5us mixed-precision roofline, rel err 1.835e-2.
"""
import sys

if "/opt/trn_rl_repo" not in sys.path:
    sys.path.insert(0, "/opt/trn_rl_repo")

import numpy as np
import ml_dtypes

import concourse.bass as bass  # noqa: F401  (registers engine classes)
import concourse.tile as tile
from concourse import bacc, mybir
from concourse import bass_utils

F32 = mybir.dt.float32
BF16 = mybir.dt.bfloat16
FP8 = mybir.dt.float8e4
Silu = mybir.ActivationFunctionType.Silu
ACT = Silu  # debug harnesses may swap to Sigmoid (CoreSim lacks Silu)
Alu = mybir.AluOpType
DR = mybir.MatmulPerfMode.DoubleRow

B, K, D = 16, 1024, 512
HS, HR, E = 2048, 1024, 8
NCORES = 8
BL = B // NCORES          # batches per core = 2
T = BL * K                # tokens per core = 2048
TT = 512                  # token tile (matmul moving dim)
NTT = T // TT             # 4 token tiles per core
NU = 2 + E                # units: 2 shared halves + 8 experts
HU = 1024                 # hidden width of every unit
NH = HU // 128            # 8 h-tiles per unit
ND = D // 128             # 4 d-tiles
NK = D // 128             # 4 contraction tiles for gate/up
WSZ = NK * HU             # per-matrix elements per partition (4096)

SX = 32.0                 # fp8 x scale
SWQ = 1024.0              # fp8 weight scale
C = 32.0                  # h-domain scale for expert fp8 h
SXW = SX * SWQ            # expert gate/up psum scale
SDC = SWQ * C             # common down psum scale (all units)

# per-unit precision: shared halves bf16, experts fp8
UNIT_FP8 = [False, False] + [True] * E
# experts first: the first unit's weights (1MB fp8 vs 3MB bf16) gate PE start
UORDER = list(range(2, NU)) + [0, 1]

# When every up-projection bias is zero (true for this problem's inputs),
# the up-branch scale and the h product fuse into one DVE op:
#   h = (ups * rwC) * silu(g).
# The general path (ts on vector + tt on gpsimd) stays available for
# nonzero biases; _run picks at call time.


def _build(fused):
    nc = bacc.Bacc("TRN2", target_bir_lowering=False, debug=False,
                   num_devices=NCORES)
    # weights packed host-side to SBUF layout: partition-contiguous, one
    # (unit, matrix) block of WSZ elements per partition per block.
    wsb = nc.dram_tensor("wsb", (128, 2 * 3 * WSZ), BF16, kind="ExternalInput")
    we8 = nc.dram_tensor("we8", (128, E * 3 * WSZ), FP8, kind="ExternalInput")
    xTb = nc.dram_tensor("xTb", (128, NK, T), BF16, kind="ExternalInput")
    xT8 = nc.dram_tensor("xT8", (128, NK, T), FP8, kind="ExternalInput")
    gb = nc.dram_tensor("gb", (128, NU, NH), F32, kind="ExternalInput")
    ub = nc.dram_tensor("ub", (128, NU, NH), F32, kind="ExternalInput")
    rw = nc.dram_tensor("rw", (128, NU, NTT), F32, kind="ExternalInput")
    cv = nc.dram_tensor("cv", (128, ND, NTT), F32, kind="ExternalInput")
    outT = nc.dram_tensor("outT", (D, T), F32, kind="ExternalOutput")

    with tile.TileContext(nc) as tc:
        with (
            tc.tile_pool(name="persist", bufs=1) as persist,
            tc.tile_pool(name="hpool", bufs=2) as hpool,
            tc.tile_pool(name="spool", bufs=2) as spool,
            tc.tile_pool(name="dpool", bufs=2) as dpool,
            tc.tile_pool(name="gups", bufs=2, space="PSUM") as gups,
            tc.tile_pool(name="ops", bufs=1, space="PSUM") as opsp,
        ):
            xb = persist.tile([128, NK, T], BF16)
            x8t = persist.tile([128, NK, T], FP8)
            gbt = persist.tile([128, NU, NH], F32)
            ubt = persist.tile([128, NU, NH], F32)
            rwt = persist.tile([128, NU, NTT], F32)
            cvt = persist.tile([128, ND, NTT], F32)

            # PE pstate warmup: dummy matmuls over a zeroed tile run during
            # the ~4us window between the engine preamble and the first
            # weight DMA landing, so the real stream starts at full clock
            # (measured: first ~20us of matmuls otherwise run at ~2x cycle
            # time). The accumulator is never read.
            warm = persist.tile([128, TT], BF16, name="warm")
            nc.vector.memset(warm[:], 0)
            wps = gups.tile([128, TT], F32, tag="g", name="warmps")
            for i in range(5):
                nc.tensor.matmul(wps[:], warm[:, 0:128], warm[:],
                                 start=(i == 0), stop=(i == 4))

            wtiles = {}
            for idx, u in enumerate(UORDER):
                fp8u = UNIT_FP8[u]
                dt_ = FP8 if fp8u else BF16
                src = we8 if fp8u else wsb
                base = (u - 2 if fp8u else u) * 3 * WSZ
                wgt = persist.tile([128, NK, HU], dt_, name=f"wg{u}")
                wut = persist.tile([128, NK, HU], dt_, name=f"wu{u}")
                wdt = persist.tile([128, NH, D], dt_, name=f"wd{u}")
                wtiles[u] = (wgt, wut, wdt)
                if idx == 0:
                    # x8 token tile 0 gates the very first matmul; tables
                    # feed the first silu/stt a few us later. Unit 0's
                    # gate/up stream by column half for an early start.
                    nc.sync.dma_start(x8t[:, :, 0:TT], xT8.ap()[:, :, 0:TT])
                    nc.sync.dma_start(gbt[:], gb.ap()[:])
                    nc.sync.dma_start(ubt[:], ub.ap()[:])
                    nc.sync.dma_start(rwt[:], rw.ap()[:])
                    nc.sync.dma_start(cvt[:], cv.ap()[:])
                    for half in range(2):
                        for k in range(NK):
                            nc.gpsimd.dma_start(
                                wgt[:, k, half * 512:(half + 1) * 512],
                                src.ap()[:, base + k * HU + half * 512:
                                          base + k * HU + (half + 1) * 512])
                        for k in range(NK):
                            nc.sync.dma_start(
                                wut[:, k, half * 512:(half + 1) * 512],
                                src.ap()[:, base + WSZ + k * HU + half * 512:
                                          base + WSZ + k * HU + (half + 1) * 512])
                    nc.gpsimd.dma_start(wdt[:],
                                        src.ap()[:, base + 2 * WSZ:base + 3 * WSZ])
                    nc.sync.dma_start(xb[:, :, 0:TT], xTb.ap()[:, :, 0:TT])
                    nc.sync.dma_start(x8t[:, :, TT:], xT8.ap()[:, :, TT:])
                    nc.sync.dma_start(xb[:, :, TT:], xTb.ap()[:, :, TT:])
                else:
                    # experts stream just-in-time on gpsimd (~1.5MB per
                    # 10.2us PE window); the late-needed shared weights ride
                    # the sync queue behind x. (Routing any weights via the
                    # scalar-engine DMA queue measured consistently ~6us
                    # slower — its transfers arbitrate poorly here.)
                    q = nc.gpsimd if fp8u else nc.sync
                    for wt, off in [(wgt, 0), (wut, WSZ), (wdt, 2 * WSZ)]:
                        q.dma_start(wt[:],
                                    src.ap()[:, base + off:base + off + WSZ])

            # The PE stream is software-pipelined by one unit: unit (t,ui)'s
            # down matmuls are emitted after unit (t,ui+1)'s gate/up, so the
            # silu->stt h-chain always has a full unit window (10-20us) of
            # PE cover instead of ~6us — this removes the pipeline-fill
            # stalls (and their pstate drops) in the first ~30us and hides
            # the t-boundary drains. Requires hpool bufs=2.
            odsts = {}

            def emit_down(t, ui, fp8u, wdt, hts):
                if ui == 0:
                    odsts[t] = [opsp.tile([128, TT], F32, tag=f"o{di}",
                                          name=f"o{di}_t{t}")
                                for di in range(ND)]
                odst = odsts[t]
                last = ui == NU - 1
                # on the very last unit of the kernel, close each d-tile's
                # accumulation group early (di-major) so the final drains
                # overlap the remaining down matmuls
                dimaj = last and t == NTT - 1
                if fp8u:
                    kds = ([(kp, di) for di in range(ND)
                            for kp in range(NH // 2)] if dimaj else
                           [(kp, di) for kp in range(NH // 2)
                            for di in range(ND)])
                    for kp, di in kds:
                        nc.tensor.matmul(
                            odst[di][:],
                            wdt[:, 2 * kp:2 * kp + 2,
                                di * 128:(di + 1) * 128],
                            hts[:, 2 * kp:2 * kp + 2, :],
                            start=(ui == 0 and kp == 0),
                            stop=(last and kp == NH // 2 - 1),
                            perf_mode=DR, skip_group_check=True)
                else:
                    kds = ([(k, di) for di in range(ND)
                            for k in range(NH)] if dimaj else
                           [(k, di) for k in range(NH)
                            for di in range(ND)])
                    for k, di in kds:
                        nc.tensor.matmul(
                            odst[di][:],
                            wdt[:, k, di * 128:(di + 1) * 128],
                            hts[:, k, :],
                            start=(ui == 0 and k == 0),
                            stop=(last and k == NH - 1),
                            skip_group_check=True)
                if last:
                    tok = slice(t * TT, (t + 1) * TT)
                    for di in range(ND):
                        dtl = dpool.tile([128, TT], F32, tag="d")
                        if di % 2 == 0:
                            nc.scalar.activation(
                                dtl[:], odst[di][:],
                                mybir.ActivationFunctionType.Identity,
                                bias=cvt[:, di, t:t + 1], scale=1.0 / SDC)
                        else:
                            nc.vector.tensor_scalar(dtl[:], odst[di][:],
                                                    1.0 / SDC,
                                                    cvt[:, di, t:t + 1],
                                                    Alu.mult, Alu.add)
                        nc.sync.dma_start(
                            outT.ap()[di * 128:(di + 1) * 128, tok], dtl[:])

            pending = None
            for t in range(NTT):
                tok = slice(t * TT, (t + 1) * TT)
                for ui, u in enumerate(UORDER):
                    fp8u = UNIT_FP8[u]
                    wgt, wut, wdt = wtiles[u]
                    sdt = BF16 if fp8u else F32
                    hts = hpool.tile([128, NH, TT], FP8 if fp8u else BF16,
                                     tag="h8" if fp8u else "hb",
                                     name=f"h_u{u}t{t}")
                    for hi in range(NH):
                        hc = slice(hi * 128, (hi + 1) * 128)
                        gps = gups.tile([128, TT], F32, tag="g")
                        if fp8u:
                            for kp in range(NK // 2):
                                nc.tensor.matmul(
                                    gps[:], wgt[:, 2 * kp:2 * kp + 2, hc],
                                    x8t[:, 2 * kp:2 * kp + 2, tok],
                                    start=(kp == 0), stop=(kp == NK // 2 - 1),
                                    perf_mode=DR)
                        else:
                            for k in range(NK):
                                nc.tensor.matmul(
                                    gps[:], wgt[:, k, hc], xb[:, k, tok],
                                    start=(k == 0), stop=(k == NK - 1))
                        ups = gups.tile([128, TT], F32, tag="u")
                        if fp8u:
                            for kp in range(NK // 2):
                                nc.tensor.matmul(
                                    ups[:], wut[:, 2 * kp:2 * kp + 2, hc],
                                    x8t[:, 2 * kp:2 * kp + 2, tok],
                                    start=(kp == 0), stop=(kp == NK // 2 - 1),
                                    perf_mode=DR)
                        else:
                            for k in range(NK):
                                nc.tensor.matmul(
                                    ups[:], wut[:, k, hc], xb[:, k, tok],
                                    start=(k == 0), stop=(k == NK - 1))
                        sg = spool.tile([128, TT], sdt, tag="sg8" if fp8u else "sgb")
                        nc.scalar.activation(sg[:], gps[:], ACT,
                                             bias=gbt[:, u, hi:hi + 1],
                                             scale=(1.0 / SXW) if fp8u else 1.0)
                        if fused:
                            nc.vector.scalar_tensor_tensor(
                                hts[:, hi, :], ups[:], rwt[:, u, t:t + 1],
                                sg[:], Alu.mult, Alu.mult)
                        else:
                            su = spool.tile([128, TT], sdt,
                                            tag="su8" if fp8u else "sub")
                            nc.vector.tensor_scalar(su[:], ups[:],
                                                    ubt[:, u, hi:hi + 1],
                                                    rwt[:, u, t:t + 1],
                                                    Alu.add, Alu.mult)
                            nc.gpsimd.tensor_tensor(hts[:, hi, :], sg[:],
                                                    su[:], Alu.mult)

                    if pending is not None:
                        emit_down(*pending)
                    pending = (t, ui, fp8u, wdt, hts)
            emit_down(*pending)
    nc.compile()
    return nc


_NC = {}


def _get_nc(fused):
    if fused not in _NC:
        _NC[fused] = _build(fused)
    return _NC[fused]


def _bf16(a):
    return np.ascontiguousarray(np.asarray(a, np.float32)).astype(ml_dtypes.bfloat16)


def _fp8(a, scale):
    return np.ascontiguousarray(
        np.asarray(a, np.float32) * scale).astype(ml_dtypes.float8_e4m3)


def _colmaj(v):
    return np.asarray(v, np.float32).reshape(-1, 128).T


def _sbufpack(w):
    """[D_in, D_out] -> [128, (D_in/128)*D_out] partition-contiguous."""
    din, dout = w.shape
    return w.reshape(din // 128, 128, dout).transpose(1, 0, 2).reshape(128, -1)


def _pack_shared(Ws_gate, bs_gate, Ws_up, bs_up, Ws_down, bs_down,
                 Wr_gate, br_gate, Wr_up, br_up, Wr_down, br_down):
    ws = np.empty((128, 2 * 3 * WSZ), np.float32)
    we = np.empty((128, E * 3 * WSZ), np.float32)
    gbt = np.empty((128, NU, NH), np.float32)
    ubt = np.empty((128, NU, NH), np.float32)
    for u in range(2):
        h0 = slice(u * HU, (u + 1) * HU)
        base = u * 3 * WSZ
        ws[:, base:base + WSZ] = _sbufpack(np.asarray(Ws_gate, np.float32)[:, h0])
        ws[:, base + WSZ:base + 2 * WSZ] = _sbufpack(
            np.asarray(Ws_up, np.float32)[:, h0])
        ws[:, base + 2 * WSZ:base + 3 * WSZ] = _sbufpack(
            np.asarray(Ws_down, np.float32)[h0, :])
        gbt[:, u, :] = _colmaj(bs_gate[h0])
        ubt[:, u, :] = _colmaj(bs_up[h0])
    for e in range(E):
        base = e * 3 * WSZ
        we[:, base:base + WSZ] = _sbufpack(np.asarray(Wr_gate, np.float32)[e])
        we[:, base + WSZ:base + 2 * WSZ] = _sbufpack(
            np.asarray(Wr_up, np.float32)[e])
        we[:, base + 2 * WSZ:base + 3 * WSZ] = _sbufpack(
            np.asarray(Wr_down, np.float32)[e])
        gbt[:, 2 + e, :] = _colmaj(br_gate[e])
        ubt[:, 2 + e, :] = _colmaj(br_up[e]) * SXW
    return _bf16(ws), _fp8(we, SWQ), gbt, ubt


def _run(inputs, trace=False):
    x = np.asarray(inputs["x"], np.float32)
    rweights = np.asarray(inputs["routing_weights"], np.float32)
    wsb, we8, gbt, ubt = _pack_shared(
        np.asarray(inputs["Ws_gate"], np.float32), inputs["bs_gate"],
        np.asarray(inputs["Ws_up"], np.float32), inputs["bs_up"],
        np.asarray(inputs["Ws_down"], np.float32), inputs["bs_down"],
        np.asarray(inputs["Wr_gate"], np.float32), inputs["br_gate"],
        np.asarray(inputs["Wr_up"], np.float32), inputs["br_up"],
        np.asarray(inputs["Wr_down"], np.float32), inputs["br_down"])
    bs_down = np.asarray(inputs["bs_down"], np.float32)
    br_down = np.asarray(inputs["br_down"], np.float32)
    # down-bias vector per batch: bs_down + sum_e rw[b,e]*br_down[e]
    cfull = bs_down[None, :] + rweights @ br_down       # [B, D]

    in_maps = []
    for i in range(NCORES):
        xT = x[BL * i:BL * (i + 1)].reshape(T, D).T     # [D, T]
        # pack x to [128, NK, T]: partition p, block k, token t = xT[k*128+p, t]
        xP = xT.reshape(NK, 128, T).transpose(1, 0, 2)
        rwtab = np.empty((128, NU, NTT), np.float32)
        rwtab[:, :2, :] = SDC
        cvtab = np.empty((128, ND, NTT), np.float32)
        for t in range(NTT):
            bg = BL * i + t // (K // TT)
            for e in range(E):
                rwtab[:, 2 + e, t] = rweights[bg, e] * (C / SXW)
            cvtab[:, :, t] = cfull[bg].reshape(ND, 128).T
        in_maps.append({"xTb": _bf16(xP), "xT8": _fp8(xP, SX),
                        "wsb": wsb, "we8": we8,
                        "gb": gbt, "ub": ubt,
                        "rw": np.ascontiguousarray(rwtab),
                        "cv": np.ascontiguousarray(cvtab)})

    fused = (not np.any(np.asarray(inputs["bs_up"], np.float32))
             and not np.any(np.asarray(inputs["br_up"], np.float32)))
    res = bass_utils.run_bass_kernel_spmd(_get_nc(fused), in_maps,
                                          core_ids=list(range(NCORES)),
                                          trace=trace)
    out = np.empty((B, K, D), np.float32)
    for i in range(NCORES):
        out[BL * i:BL * (i + 1)] = res.results[i]["outT"].T.reshape(BL, K, D)
    return out, res


def kernel(**inputs) -> np.ndarray:
    out, _ = _run(inputs, trace=False)
    return out


# revision 61
# speedup vs baseline: 1.1930x; 1.0058x over previous
"""MoE FFN (shared SwiGLU + 8 dense-routed SwiGLU experts) on 8 TRN2 NeuronCores.

Strategy: data-parallel over batch (B=16 -> 2 batches/core). The 10 uniform
512->1024->512 SwiGLU units (2 shared halves + 8 experts) run with per-unit
precision: shared units in bf16, expert units in fp8e4m3 DoubleRow matmuls
(2x PE throughput). Expert quantization errors are independent across the 8
experts and average down under the routing weights, keeping total rel err
~1.8e-2 (<2e-2 gate) while cutting PE time to ~0.6x of the bf16 roofline.

All weights stay resident in SBUF (~170KB/partition incl. x), host-repacked
to partition-contiguous layout so the whole working set loads with ~50 large
DMA descriptors (an earlier revision spent 712us of gpsimd time issuing 960
fine-grained weight DMAs). Loop is token-tile-outer / unit-inner so all 10
units' down-projections accumulate directly in PSUM at a common scale
(shared h pre-scaled by SW*C=32768, exact in floating point); one op per
(d-tile, token-tile) drains PSUM with bias + descale fused, alternating
between the scalar and vector engines. With zero up-biases (this problem's
inputs) the whole up-branch scale + h product is a single fused
scalar_tensor_tensor on the DVE, so each h-tile costs one scalar-engine op
(silu) and one DVE op. Measured: 525us vs the 853us bf16 baseline; PE busy
~500us vs the 491.5us mixed-precision roofline, rel err 1.835e-2.
"""
import sys

if "/opt/trn_rl_repo" not in sys.path:
    sys.path.insert(0, "/opt/trn_rl_repo")

import numpy as np
import ml_dtypes

import concourse.bass as bass  # noqa: F401  (registers engine classes)
import concourse.tile as tile
from concourse import bacc, mybir
from concourse import bass_utils

F32 = mybir.dt.float32
BF16 = mybir.dt.bfloat16
FP8 = mybir.dt.float8e4
Silu = mybir.ActivationFunctionType.Silu
ACT = Silu  # debug harnesses may swap to Sigmoid (CoreSim lacks Silu)
Alu = mybir.AluOpType
DR = mybir.MatmulPerfMode.DoubleRow

B, K, D = 16, 1024, 512
HS, HR, E = 2048, 1024, 8
NCORES = 8
BL = B // NCORES          # batches per core = 2
T = BL * K                # tokens per core = 2048
TT = 512                  # token tile (matmul moving dim)
NTT = T // TT             # 4 token tiles per core
NU = 2 + E                # units: 2 shared halves + 8 experts
HU = 1024                 # hidden width of every unit
NH = HU // 128            # 8 h-tiles per unit
ND = D // 128             # 4 d-tiles
NK = D // 128             # 4 contraction tiles for gate/up
WSZ = NK * HU             # per-matrix elements per partition (4096)

SX = 32.0                 # fp8 x scale
SWQ = 1024.0              # fp8 weight scale
C = 32.0                  # h-domain scale for expert fp8 h
SXW = SX * SWQ            # expert gate/up psum scale
SDC = SWQ * C             # common down psum scale (all units)

# per-unit precision: shared halves bf16, experts fp8
UNIT_FP8 = [False, False] + [True] * E
# experts first: the first unit's weights (1MB fp8 vs 3MB bf16) gate PE start
UORDER = list(range(2, NU)) + [0, 1]

# When every up-projection bias is zero (true for this problem's inputs),
# the up-branch scale and the h product fuse into one DVE op:
#   h = (ups * rwC) * silu(g).
# The general path (ts on vector + tt on gpsimd) stays available for
# nonzero biases; _run picks at call time.


def _build(fused):
    nc = bacc.Bacc("TRN2", target_bir_lowering=False, debug=False,
                   num_devices=NCORES)
    # weights packed host-side to SBUF layout: partition-contiguous, one
    # (unit, matrix) block of WSZ elements per partition per block.
    wsb = nc.dram_tensor("wsb", (128, 2 * 3 * WSZ), BF16, kind="ExternalInput")
    we8 = nc.dram_tensor("we8", (128, E * 3 * WSZ), FP8, kind="ExternalInput")
    xTb = nc.dram_tensor("xTb", (128, NK, T), BF16, kind="ExternalInput")
    xT8 = nc.dram_tensor("xT8", (128, NK, T), FP8, kind="ExternalInput")
    gb = nc.dram_tensor("gb", (128, NU, NH), F32, kind="ExternalInput")
    ub = nc.dram_tensor("ub", (128, NU, NH), F32, kind="ExternalInput")
    rw = nc.dram_tensor("rw", (128, NU, NTT), F32, kind="ExternalInput")
    cv = nc.dram_tensor("cv", (128, ND, NTT), F32, kind="ExternalInput")
    outT = nc.dram_tensor("outT", (D, T), F32, kind="ExternalOutput")

    with tile.TileContext(nc) as tc:
        with (
            tc.tile_pool(name="persist", bufs=1) as persist,
            tc.tile_pool(name="hpool", bufs=2) as hpool,
            tc.tile_pool(name="spool", bufs=2) as spool,
            tc.tile_pool(name="dpool", bufs=2) as dpool,
            tc.tile_pool(name="gups", bufs=2, space="PSUM") as gups,
            tc.tile_pool(name="ops", bufs=1, space="PSUM") as opsp,
        ):
            xb = persist.tile([128, NK, T], BF16)
            x8t = persist.tile([128, NK, T], FP8)
            gbt = persist.tile([128, NU, NH], F32)
            ubt = persist.tile([128, NU, NH], F32)
            rwt = persist.tile([128, NU, NTT], F32)
            cvt = persist.tile([128, ND, NTT], F32)

            # PE pstate warmup: dummy matmuls over a zeroed tile run during
            # the ~4us window between the engine preamble and the first
            # weight DMA landing, so the real stream starts at full clock
            # (measured: first ~20us of matmuls otherwise run at ~2x cycle
            # time). The accumulator is never read.
            warm = persist.tile([128, TT], BF16, name="warm")
            nc.vector.memset(warm[:], 0)
            wps = gups.tile([128, TT], F32, tag="g", name="warmps")
            for i in range(5):
                nc.tensor.matmul(wps[:], warm[:, 0:128], warm[:],
                                 start=(i == 0), stop=(i == 4))

            wtiles = {}
            for idx, u in enumerate(UORDER):
                fp8u = UNIT_FP8[u]
                dt_ = FP8 if fp8u else BF16
                src = we8 if fp8u else wsb
                base = (u - 2 if fp8u else u) * 3 * WSZ
                wgt = persist.tile([128, NK, HU], dt_, name=f"wg{u}")
                wut = persist.tile([128, NK, HU], dt_, name=f"wu{u}")
                wdt = persist.tile([128, NH, D], dt_, name=f"wd{u}")
                wtiles[u] = (wgt, wut, wdt)
                if idx == 0:
                    # x8 token tile 0 gates the very first matmul — split by
                    # k-pair so that matmul only waits on the 128KB it reads.
                    # Tables feed the first silu/stt a few us later. Unit 0's
                    # gate/up stream by column half for an early start.
                    nc.sync.dma_start(x8t[:, 0:2, 0:TT], xT8.ap()[:, 0:2, 0:TT])
                    nc.sync.dma_start(x8t[:, 2:4, 0:TT], xT8.ap()[:, 2:4, 0:TT])
                    nc.sync.dma_start(gbt[:], gb.ap()[:])
                    nc.sync.dma_start(ubt[:], ub.ap()[:])
                    nc.sync.dma_start(rwt[:], rw.ap()[:])
                    nc.sync.dma_start(cvt[:], cv.ap()[:])
                    for half in range(2):
                        for k in range(NK):
                            nc.gpsimd.dma_start(
                                wgt[:, k, half * 512:(half + 1) * 512],
                                src.ap()[:, base + k * HU + half * 512:
                                          base + k * HU + (half + 1) * 512])
                        for k in range(NK):
                            nc.sync.dma_start(
                                wut[:, k, half * 512:(half + 1) * 512],
                                src.ap()[:, base + WSZ + k * HU + half * 512:
                                          base + WSZ + k * HU + (half + 1) * 512])
                    nc.gpsimd.dma_start(wdt[:],
                                        src.ap()[:, base + 2 * WSZ:base + 3 * WSZ])
                    nc.sync.dma_start(xb[:, :, 0:TT], xTb.ap()[:, :, 0:TT])
                    nc.sync.dma_start(x8t[:, :, TT:], xT8.ap()[:, :, TT:])
                    nc.sync.dma_start(xb[:, :, TT:], xTb.ap()[:, :, TT:])
                else:
                    # experts stream just-in-time on gpsimd (~1.5MB per
                    # 10.2us PE window); the late-needed shared weights ride
                    # the sync queue behind x. (Routing any weights via the
                    # scalar-engine DMA queue measured consistently ~6us
                    # slower — its transfers arbitrate poorly here.)
                    q = nc.gpsimd if fp8u else nc.sync
                    for wt, off in [(wgt, 0), (wut, WSZ), (wdt, 2 * WSZ)]:
                        q.dma_start(wt[:],
                                    src.ap()[:, base + off:base + off + WSZ])

            # The PE stream is software-pipelined by one unit: unit (t,ui)'s
            # down matmuls are emitted after unit (t,ui+1)'s gate/up, so the
            # silu->stt h-chain always has a full unit window (10-20us) of
            # PE cover instead of ~6us — this removes the pipeline-fill
            # stalls (and their pstate drops) in the first ~30us and hides
            # the t-boundary drains. Requires hpool bufs=2.
            odsts = {}

            def emit_down(t, ui, fp8u, wdt, hts):
                if ui == 0:
                    odsts[t] = [opsp.tile([128, TT], F32, tag=f"o{di}",
                                          name=f"o{di}_t{t}")
                                for di in range(ND)]
                odst = odsts[t]
                last = ui == NU - 1
                # on the very last unit of the kernel, close each d-tile's
                # accumulation group early (di-major) so the final drains
                # overlap the remaining down matmuls
                dimaj = last and t == NTT - 1
                if fp8u:
                    kds = ([(kp, di) for di in range(ND)
                            for kp in range(NH // 2)] if dimaj else
                           [(kp, di) for kp in range(NH // 2)
                            for di in range(ND)])
                    for kp, di in kds:
                        nc.tensor.matmul(
                            odst[di][:],
                            wdt[:, 2 * kp:2 * kp + 2,
                                di * 128:(di + 1) * 128],
                            hts[:, 2 * kp:2 * kp + 2, :],
                            start=(ui == 0 and kp == 0),
                            stop=(last and kp == NH // 2 - 1),
                            perf_mode=DR, skip_group_check=True)
                else:
                    kds = ([(k, di) for di in range(ND)
                            for k in range(NH)] if dimaj else
                           [(k, di) for k in range(NH)
                            for di in range(ND)])
                    for k, di in kds:
                        nc.tensor.matmul(
                            odst[di][:],
                            wdt[:, k, di * 128:(di + 1) * 128],
                            hts[:, k, :],
                            start=(ui == 0 and k == 0),
                            stop=(last and k == NH - 1),
                            skip_group_check=True)
                if last:
                    tok = slice(t * TT, (t + 1) * TT)
                    for di in range(ND):
                        dtl = dpool.tile([128, TT], F32, tag="d")
                        if di % 2 == 0:
                            nc.scalar.activation(
                                dtl[:], odst[di][:],
                                mybir.ActivationFunctionType.Identity,
                                bias=cvt[:, di, t:t + 1], scale=1.0 / SDC)
                        else:
                            nc.vector.tensor_scalar(dtl[:], odst[di][:],
                                                    1.0 / SDC,
                                                    cvt[:, di, t:t + 1],
                                                    Alu.mult, Alu.add)
                        nc.sync.dma_start(
                            outT.ap()[di * 128:(di + 1) * 128, tok], dtl[:])

            pending = None
            for t in range(NTT):
                tok = slice(t * TT, (t + 1) * TT)
                for ui, u in enumerate(UORDER):
                    fp8u = UNIT_FP8[u]
                    wgt, wut, wdt = wtiles[u]
                    sdt = BF16 if fp8u else F32
                    hts = hpool.tile([128, NH, TT], FP8 if fp8u else BF16,
                                     tag="h8" if fp8u else "hb",
                                     name=f"h_u{u}t{t}")
                    for hi in range(NH):
                        hc = slice(hi * 128, (hi + 1) * 128)
                        gps = gups.tile([128, TT], F32, tag="g")
                        if fp8u:
                            for kp in range(NK // 2):
                                nc.tensor.matmul(
                                    gps[:], wgt[:, 2 * kp:2 * kp + 2, hc],
                                    x8t[:, 2 * kp:2 * kp + 2, tok],
                                    start=(kp == 0), stop=(kp == NK // 2 - 1),
                                    perf_mode=DR)
                        else:
                            for k in range(NK):
                                nc.tensor.matmul(
                                    gps[:], wgt[:, k, hc], xb[:, k, tok],
                                    start=(k == 0), stop=(k == NK - 1))
                        ups = gups.tile([128, TT], F32, tag="u")
                        if fp8u:
                            for kp in range(NK // 2):
                                nc.tensor.matmul(
                                    ups[:], wut[:, 2 * kp:2 * kp + 2, hc],
                                    x8t[:, 2 * kp:2 * kp + 2, tok],
                                    start=(kp == 0), stop=(kp == NK // 2 - 1),
                                    perf_mode=DR)
                        else:
                            for k in range(NK):
                                nc.tensor.matmul(
                                    ups[:], wut[:, k, hc], xb[:, k, tok],
                                    start=(k == 0), stop=(k == NK - 1))
                        sg = spool.tile([128, TT], sdt, tag="sg8" if fp8u else "sgb")
                        nc.scalar.activation(sg[:], gps[:], ACT,
                                             bias=gbt[:, u, hi:hi + 1],
                                             scale=(1.0 / SXW) if fp8u else 1.0)
                        if fused:
                            nc.vector.scalar_tensor_tensor(
                                hts[:, hi, :], ups[:], rwt[:, u, t:t + 1],
                                sg[:], Alu.mult, Alu.mult)
                        else:
                            su = spool.tile([128, TT], sdt,
                                            tag="su8" if fp8u else "sub")
                            nc.vector.tensor_scalar(su[:], ups[:],
                                                    ubt[:, u, hi:hi + 1],
                                                    rwt[:, u, t:t + 1],
                                                    Alu.add, Alu.mult)
                            nc.gpsimd.tensor_tensor(hts[:, hi, :], sg[:],
                                                    su[:], Alu.mult)

                    if pending is not None:
                        emit_down(*pending)
                    pending = (t, ui, fp8u, wdt, hts)
            emit_down(*pending)
    nc.compile()
    return nc


_NC = {}


def _get_nc(fused):
    if fused not in _NC:
        _NC[fused] = _build(fused)
    return _NC[fused]


def _bf16(a):
    return np.ascontiguousarray(np.asarray(a, np.float32)).astype(ml_dtypes.bfloat16)


def _fp8(a, scale):
    return np.ascontiguousarray(
        np.asarray(a, np.float32) * scale).astype(ml_dtypes.float8_e4m3)


def _colmaj(v):
    return np.asarray(v, np.float32).reshape(-1, 128).T


def _sbufpack(w):
    """[D_in, D_out] -> [128, (D_in/128)*D_out] partition-contiguous."""
    din, dout = w.shape
    return w.reshape(din // 128, 128, dout).transpose(1, 0, 2).reshape(128, -1)


def _pack_shared(Ws_gate, bs_gate, Ws_up, bs_up, Ws_down, bs_down,
                 Wr_gate, br_gate, Wr_up, br_up, Wr_down, br_down):
    ws = np.empty((128, 2 * 3 * WSZ), np.float32)
    we = np.empty((128, E * 3 * WSZ), np.float32)
    gbt = np.empty((128, NU, NH), np.float32)
    ubt = np.empty((128, NU, NH), np.float32)
    for u in range(2):
        h0 = slice(u * HU, (u + 1) * HU)
        base = u * 3 * WSZ
        ws[:, base:base + WSZ] = _sbufpack(np.asarray(Ws_gate, np.float32)[:, h0])
        ws[:, base + WSZ:base + 2 * WSZ] = _sbufpack(
            np.asarray(Ws_up, np.float32)[:, h0])
        ws[:, base + 2 * WSZ:base + 3 * WSZ] = _sbufpack(
            np.asarray(Ws_down, np.float32)[h0, :])
        gbt[:, u, :] = _colmaj(bs_gate[h0])
        ubt[:, u, :] = _colmaj(bs_up[h0])
    for e in range(E):
        base = e * 3 * WSZ
        we[:, base:base + WSZ] = _sbufpack(np.asarray(Wr_gate, np.float32)[e])
        we[:, base + WSZ:base + 2 * WSZ] = _sbufpack(
            np.asarray(Wr_up, np.float32)[e])
        we[:, base + 2 * WSZ:base + 3 * WSZ] = _sbufpack(
            np.asarray(Wr_down, np.float32)[e])
        gbt[:, 2 + e, :] = _colmaj(br_gate[e])
        ubt[:, 2 + e, :] = _colmaj(br_up[e]) * SXW
    return _bf16(ws), _fp8(we, SWQ), gbt, ubt


def _run(inputs, trace=False):
    x = np.asarray(inputs["x"], np.float32)
    rweights = np.asarray(inputs["routing_weights"], np.float32)
    wsb, we8, gbt, ubt = _pack_shared(
        np.asarray(inputs["Ws_gate"], np.float32), inputs["bs_gate"],
        np.asarray(inputs["Ws_up"], np.float32), inputs["bs_up"],
        np.asarray(inputs["Ws_down"], np.float32), inputs["bs_down"],
        np.asarray(inputs["Wr_gate"], np.float32), inputs["br_gate"],
        np.asarray(inputs["Wr_up"], np.float32), inputs["br_up"],
        np.asarray(inputs["Wr_down"], np.float32), inputs["br_down"])
    bs_down = np.asarray(inputs["bs_down"], np.float32)
    br_down = np.asarray(inputs["br_down"], np.float32)
    # down-bias vector per batch: bs_down + sum_e rw[b,e]*br_down[e]
    cfull = bs_down[None, :] + rweights @ br_down       # [B, D]

    in_maps = []
    for i in range(NCORES):
        xT = x[BL * i:BL * (i + 1)].reshape(T, D).T     # [D, T]
        # pack x to [128, NK, T]: partition p, block k, token t = xT[k*128+p, t]
        xP = xT.reshape(NK, 128, T).transpose(1, 0, 2)
        rwtab = np.empty((128, NU, NTT), np.float32)
        rwtab[:, :2, :] = SDC
        cvtab = np.empty((128, ND, NTT), np.float32)
        for t in range(NTT):
            bg = BL * i + t // (K // TT)
            for e in range(E):
                rwtab[:, 2 + e, t] = rweights[bg, e] * (C / SXW)
            cvtab[:, :, t] = cfull[bg].reshape(ND, 128).T
        in_maps.append({"xTb": _bf16(xP), "xT8": _fp8(xP, SX),
                        "wsb": wsb, "we8": we8,
                        "gb": gbt, "ub": ubt,
                        "rw": np.ascontiguousarray(rwtab),
                        "cv": np.ascontiguousarray(cvtab)})

    fused = (not np.any(np.asarray(inputs["bs_up"], np.float32))
             and not np.any(np.asarray(inputs["br_up"], np.float32)))
    res = bass_utils.run_bass_kernel_spmd(_get_nc(fused), in_maps,
                                          core_ids=list(range(NCORES)),
                                          trace=trace)
    out = np.empty((B, K, D), np.float32)
    for i in range(NCORES):
        out[BL * i:BL * (i + 1)] = res.results[i]["outT"].T.reshape(BL, K, D)
    return out, res


def kernel(**inputs) -> np.ndarray:
    out, _ = _run(inputs, trace=False)
    return out


# revision 62
# speedup vs baseline: 1.1933x; 1.0003x over previous
"""MoE FFN (shared SwiGLU + 8 dense-routed SwiGLU experts) on 8 TRN2 NeuronCores.

Strategy: data-parallel over batch (B=16 -> 2 batches/core). The 10 uniform
512->1024->512 SwiGLU units (2 shared halves + 8 experts) run with per-unit
precision: shared units in bf16, expert units in fp8e4m3 DoubleRow matmuls
(2x PE throughput). Expert quantization errors are independent across the 8
experts and average down under the routing weights, keeping total rel err
~1.8e-2 (<2e-2 gate) while cutting PE time to ~0.6x of the bf16 roofline.

All weights stay resident in SBUF (~170KB/partition incl. x), host-repacked
to partition-contiguous layout so the whole working set loads with ~50 large
DMA descriptors (an earlier revision spent 712us of gpsimd time issuing 960
fine-grained weight DMAs). Loop is token-tile-outer / unit-inner so all 10
units' down-projections accumulate directly in PSUM at a common scale
(shared h pre-scaled by SW*C=32768, exact in floating point); one op per
(d-tile, token-tile) drains PSUM with bias + descale fused, alternating
between the scalar and vector engines. With zero up-biases (this problem's
inputs) the whole up-branch scale + h product is a single fused
scalar_tensor_tensor on the DVE, so each h-tile costs one scalar-engine op
(silu) and one DVE op. Measured: 525us vs the 853us bf16 baseline; PE busy
~500us vs the 491.5us mixed-precision roofline, rel err 1.835e-2.
"""
import sys

if "/opt/trn_rl_repo" not in sys.path:
    sys.path.insert(0, "/opt/trn_rl_repo")

import numpy as np
import ml_dtypes

import concourse.bass as bass  # noqa: F401  (registers engine classes)
import concourse.tile as tile
from concourse import bacc, mybir
from concourse import bass_utils

F32 = mybir.dt.float32
BF16 = mybir.dt.bfloat16
FP8 = mybir.dt.float8e4
Silu = mybir.ActivationFunctionType.Silu
ACT = Silu  # debug harnesses may swap to Sigmoid (CoreSim lacks Silu)
Alu = mybir.AluOpType
DR = mybir.MatmulPerfMode.DoubleRow

B, K, D = 16, 1024, 512
HS, HR, E = 2048, 1024, 8
NCORES = 8
BL = B // NCORES          # batches per core = 2
T = BL * K                # tokens per core = 2048
TT = 512                  # token tile (matmul moving dim)
NTT = T // TT             # 4 token tiles per core
NU = 2 + E                # units: 2 shared halves + 8 experts
HU = 1024                 # hidden width of every unit
NH = HU // 128            # 8 h-tiles per unit
ND = D // 128             # 4 d-tiles
NK = D // 128             # 4 contraction tiles for gate/up
WSZ = NK * HU             # per-matrix elements per partition (4096)

SX = 32.0                 # fp8 x scale
SWQ = 1024.0              # fp8 weight scale
C = 32.0                  # h-domain scale for expert fp8 h
SXW = SX * SWQ            # expert gate/up psum scale
SDC = SWQ * C             # common down psum scale (all units)

# per-unit precision: shared halves bf16, experts fp8
UNIT_FP8 = [False, False] + [True] * E
# experts first: the first unit's weights (1MB fp8 vs 3MB bf16) gate PE start
UORDER = list(range(2, NU)) + [0, 1]

# When every up-projection bias is zero (true for this problem's inputs),
# the up-branch scale and the h product fuse into one DVE op:
#   h = (ups * rwC) * silu(g).
# The general path (ts on vector + tt on gpsimd) stays available for
# nonzero biases; _run picks at call time.


def _build(fused):
    nc = bacc.Bacc("TRN2", target_bir_lowering=False, debug=False,
                   num_devices=NCORES)
    # weights packed host-side to SBUF layout: partition-contiguous, one
    # (unit, matrix) block of WSZ elements per partition per block.
    wsb = nc.dram_tensor("wsb", (128, 2 * 3 * WSZ), BF16, kind="ExternalInput")
    we8 = nc.dram_tensor("we8", (128, E * 3 * WSZ), FP8, kind="ExternalInput")
    xTb = nc.dram_tensor("xTb", (128, NK, T), BF16, kind="ExternalInput")
    xT8 = nc.dram_tensor("xT8", (128, NK, T), FP8, kind="ExternalInput")
    gb = nc.dram_tensor("gb", (128, NU, NH), F32, kind="ExternalInput")
    ub = nc.dram_tensor("ub", (128, NU, NH), F32, kind="ExternalInput")
    rw = nc.dram_tensor("rw", (128, NU, NTT), F32, kind="ExternalInput")
    cv = nc.dram_tensor("cv", (128, ND, NTT), F32, kind="ExternalInput")
    outT = nc.dram_tensor("outT", (D, T), F32, kind="ExternalOutput")

    with tile.TileContext(nc) as tc:
        with (
            tc.tile_pool(name="persist", bufs=1) as persist,
            tc.tile_pool(name="hpool", bufs=2) as hpool,
            tc.tile_pool(name="spool", bufs=2) as spool,
            tc.tile_pool(name="dpool", bufs=2) as dpool,
            tc.tile_pool(name="gups", bufs=2, space="PSUM") as gups,
            tc.tile_pool(name="ops", bufs=1, space="PSUM") as opsp,
        ):
            xb = persist.tile([128, NK, T], BF16)
            x8t = persist.tile([128, NK, T], FP8)
            gbt = persist.tile([128, NU, NH], F32)
            ubt = persist.tile([128, NU, NH], F32)
            rwt = persist.tile([128, NU, NTT], F32)
            cvt = persist.tile([128, ND, NTT], F32)

            # PE pstate warmup: dummy matmuls over a zeroed tile run during
            # the ~4us window between the engine preamble and the first
            # weight DMA landing, so the real stream starts at full clock
            # (measured: first ~20us of matmuls otherwise run at ~2x cycle
            # time). The accumulator is never read.
            warm = persist.tile([128, TT], BF16, name="warm")
            wout = persist.tile([128, TT], BF16, name="wout")
            nc.vector.memset(warm[:], 0)
            wps = gups.tile([128, TT], F32, tag="g", name="warmps")
            for i in range(5):
                nc.tensor.matmul(wps[:], warm[:, 0:128], warm[:],
                                 start=(i == 0), stop=(i == 4))
            # pre-warm the scalar and vector pipelines too: the first real
            # silu/stt otherwise pay multi-us cold-engine latency that stalls
            # the first unit's down matmuls
            for _ in range(2):
                nc.scalar.activation(wout[:], warm[:], ACT)
                nc.vector.scalar_tensor_tensor(wout[:], warm[:], 1.0,
                                               warm[:], Alu.mult, Alu.mult)

            wtiles = {}
            for idx, u in enumerate(UORDER):
                fp8u = UNIT_FP8[u]
                dt_ = FP8 if fp8u else BF16
                src = we8 if fp8u else wsb
                base = (u - 2 if fp8u else u) * 3 * WSZ
                wgt = persist.tile([128, NK, HU], dt_, name=f"wg{u}")
                wut = persist.tile([128, NK, HU], dt_, name=f"wu{u}")
                wdt = persist.tile([128, NH, D], dt_, name=f"wd{u}")
                wtiles[u] = (wgt, wut, wdt)
                if idx == 0:
                    # x8 token tile 0 gates the very first matmul — split by
                    # k-pair so that matmul only waits on the 128KB it reads.
                    # Tables feed the first silu/stt a few us later. Unit 0's
                    # gate/up stream by column half for an early start.
                    nc.sync.dma_start(x8t[:, 0:2, 0:TT], xT8.ap()[:, 0:2, 0:TT])
                    nc.sync.dma_start(x8t[:, 2:4, 0:TT], xT8.ap()[:, 2:4, 0:TT])
                    nc.sync.dma_start(gbt[:], gb.ap()[:])
                    nc.sync.dma_start(ubt[:], ub.ap()[:])
                    nc.sync.dma_start(rwt[:], rw.ap()[:])
                    nc.sync.dma_start(cvt[:], cv.ap()[:])
                    for half in range(2):
                        for k in range(NK):
                            nc.gpsimd.dma_start(
                                wgt[:, k, half * 512:(half + 1) * 512],
                                src.ap()[:, base + k * HU + half * 512:
                                          base + k * HU + (half + 1) * 512])
                        for k in range(NK):
                            nc.sync.dma_start(
                                wut[:, k, half * 512:(half + 1) * 512],
                                src.ap()[:, base + WSZ + k * HU + half * 512:
                                          base + WSZ + k * HU + (half + 1) * 512])
                    nc.gpsimd.dma_start(wdt[:],
                                        src.ap()[:, base + 2 * WSZ:base + 3 * WSZ])
                    nc.sync.dma_start(xb[:, :, 0:TT], xTb.ap()[:, :, 0:TT])
                    nc.sync.dma_start(x8t[:, :, TT:], xT8.ap()[:, :, TT:])
                    nc.sync.dma_start(xb[:, :, TT:], xTb.ap()[:, :, TT:])
                else:
                    # experts stream just-in-time on gpsimd (~1.5MB per
                    # 10.2us PE window); the late-needed shared weights ride
                    # the sync queue behind x. (Routing any weights via the
                    # scalar-engine DMA queue measured consistently ~6us
                    # slower — its transfers arbitrate poorly here.)
                    q = nc.gpsimd if fp8u else nc.sync
                    for wt, off in [(wgt, 0), (wut, WSZ), (wdt, 2 * WSZ)]:
                        q.dma_start(wt[:],
                                    src.ap()[:, base + off:base + off + WSZ])

            # The PE stream is software-pipelined by one unit: unit (t,ui)'s
            # down matmuls are emitted after unit (t,ui+1)'s gate/up, so the
            # silu->stt h-chain always has a full unit window (10-20us) of
            # PE cover instead of ~6us — this removes the pipeline-fill
            # stalls (and their pstate drops) in the first ~30us and hides
            # the t-boundary drains. Requires hpool bufs=2.
            odsts = {}

            def emit_down(t, ui, fp8u, wdt, hts):
                if ui == 0:
                    odsts[t] = [opsp.tile([128, TT], F32, tag=f"o{di}",
                                          name=f"o{di}_t{t}")
                                for di in range(ND)]
                odst = odsts[t]
                last = ui == NU - 1
                # on the very last unit of the kernel, close each d-tile's
                # accumulation group early (di-major) so the final drains
                # overlap the remaining down matmuls
                dimaj = last and t == NTT - 1
                if fp8u:
                    kds = ([(kp, di) for di in range(ND)
                            for kp in range(NH // 2)] if dimaj else
                           [(kp, di) for kp in range(NH // 2)
                            for di in range(ND)])
                    for kp, di in kds:
                        nc.tensor.matmul(
                            odst[di][:],
                            wdt[:, 2 * kp:2 * kp + 2,
                                di * 128:(di + 1) * 128],
                            hts[:, 2 * kp:2 * kp + 2, :],
                            start=(ui == 0 and kp == 0),
                            stop=(last and kp == NH // 2 - 1),
                            perf_mode=DR, skip_group_check=True)
                else:
                    kds = ([(k, di) for di in range(ND)
                            for k in range(NH)] if dimaj else
                           [(k, di) for k in range(NH)
                            for di in range(ND)])
                    for k, di in kds:
                        nc.tensor.matmul(
                            odst[di][:],
                            wdt[:, k, di * 128:(di + 1) * 128],
                            hts[:, k, :],
                            start=(ui == 0 and k == 0),
                            stop=(last and k == NH - 1),
                            skip_group_check=True)
                if last:
                    tok = slice(t * TT, (t + 1) * TT)
                    for di in range(ND):
                        dtl = dpool.tile([128, TT], F32, tag="d")
                        if di % 2 == 0:
                            nc.scalar.activation(
                                dtl[:], odst[di][:],
                                mybir.ActivationFunctionType.Identity,
                                bias=cvt[:, di, t:t + 1], scale=1.0 / SDC)
                        else:
                            nc.vector.tensor_scalar(dtl[:], odst[di][:],
                                                    1.0 / SDC,
                                                    cvt[:, di, t:t + 1],
                                                    Alu.mult, Alu.add)
                        nc.sync.dma_start(
                            outT.ap()[di * 128:(di + 1) * 128, tok], dtl[:])

            pending = None
            for t in range(NTT):
                tok = slice(t * TT, (t + 1) * TT)
                for ui, u in enumerate(UORDER):
                    fp8u = UNIT_FP8[u]
                    wgt, wut, wdt = wtiles[u]
                    sdt = BF16 if fp8u else F32
                    hts = hpool.tile([128, NH, TT], FP8 if fp8u else BF16,
                                     tag="h8" if fp8u else "hb",
                                     name=f"h_u{u}t{t}")
                    for hi in range(NH):
                        hc = slice(hi * 128, (hi + 1) * 128)
                        gps = gups.tile([128, TT], F32, tag="g")
                        if fp8u:
                            for kp in range(NK // 2):
                                nc.tensor.matmul(
                                    gps[:], wgt[:, 2 * kp:2 * kp + 2, hc],
                                    x8t[:, 2 * kp:2 * kp + 2, tok],
                                    start=(kp == 0), stop=(kp == NK // 2 - 1),
                                    perf_mode=DR)
                        else:
                            for k in range(NK):
                                nc.tensor.matmul(
                                    gps[:], wgt[:, k, hc], xb[:, k, tok],
                                    start=(k == 0), stop=(k == NK - 1))
                        ups = gups.tile([128, TT], F32, tag="u")
                        if fp8u:
                            for kp in range(NK // 2):
                                nc.tensor.matmul(
                                    ups[:], wut[:, 2 * kp:2 * kp + 2, hc],
                                    x8t[:, 2 * kp:2 * kp + 2, tok],
                                    start=(kp == 0), stop=(kp == NK // 2 - 1),
                                    perf_mode=DR)
                        else:
                            for k in range(NK):
                                nc.tensor.matmul(
                                    ups[:], wut[:, k, hc], xb[:, k, tok],
                                    start=(k == 0), stop=(k == NK - 1))
                        sg = spool.tile([128, TT], sdt, tag="sg8" if fp8u else "sgb")
                        nc.scalar.activation(sg[:], gps[:], ACT,
                                             bias=gbt[:, u, hi:hi + 1],
                                             scale=(1.0 / SXW) if fp8u else 1.0)
                        if fused:
                            nc.vector.scalar_tensor_tensor(
                                hts[:, hi, :], ups[:], rwt[:, u, t:t + 1],
                                sg[:], Alu.mult, Alu.mult)
                        else:
                            su = spool.tile([128, TT], sdt,
                                            tag="su8" if fp8u else "sub")
                            nc.vector.tensor_scalar(su[:], ups[:],
                                                    ubt[:, u, hi:hi + 1],
                                                    rwt[:, u, t:t + 1],
                                                    Alu.add, Alu.mult)
                            nc.gpsimd.tensor_tensor(hts[:, hi, :], sg[:],
                                                    su[:], Alu.mult)

                    if pending is not None:
                        emit_down(*pending)
                    pending = (t, ui, fp8u, wdt, hts)
            emit_down(*pending)
    nc.compile()
    return nc


_NC = {}


def _get_nc(fused):
    if fused not in _NC:
        _NC[fused] = _build(fused)
    return _NC[fused]


def _bf16(a):
    return np.ascontiguousarray(np.asarray(a, np.float32)).astype(ml_dtypes.bfloat16)


def _fp8(a, scale):
    return np.ascontiguousarray(
        np.asarray(a, np.float32) * scale).astype(ml_dtypes.float8_e4m3)


def _colmaj(v):
    return np.asarray(v, np.float32).reshape(-1, 128).T


def _sbufpack(w):
    """[D_in, D_out] -> [128, (D_in/128)*D_out] partition-contiguous."""
    din, dout = w.shape
    return w.reshape(din // 128, 128, dout).transpose(1, 0, 2).reshape(128, -1)


def _pack_shared(Ws_gate, bs_gate, Ws_up, bs_up, Ws_down, bs_down,
                 Wr_gate, br_gate, Wr_up, br_up, Wr_down, br_down):
    ws = np.empty((128, 2 * 3 * WSZ), np.float32)
    we = np.empty((128, E * 3 * WSZ), np.float32)
    gbt = np.empty((128, NU, NH), np.float32)
    ubt = np.empty((128, NU, NH), np.float32)
    for u in range(2):
        h0 = slice(u * HU, (u + 1) * HU)
        base = u * 3 * WSZ
        ws[:, base:base + WSZ] = _sbufpack(np.asarray(Ws_gate, np.float32)[:, h0])
        ws[:, base + WSZ:base + 2 * WSZ] = _sbufpack(
            np.asarray(Ws_up, np.float32)[:, h0])
        ws[:, base + 2 * WSZ:base + 3 * WSZ] = _sbufpack(
            np.asarray(Ws_down, np.float32)[h0, :])
        gbt[:, u, :] = _colmaj(bs_gate[h0])
        ubt[:, u, :] = _colmaj(bs_up[h0])
    for e in range(E):
        base = e * 3 * WSZ
        we[:, base:base + WSZ] = _sbufpack(np.asarray(Wr_gate, np.float32)[e])
        we[:, base + WSZ:base + 2 * WSZ] = _sbufpack(
            np.asarray(Wr_up, np.float32)[e])
        we[:, base + 2 * WSZ:base + 3 * WSZ] = _sbufpack(
            np.asarray(Wr_down, np.float32)[e])
        gbt[:, 2 + e, :] = _colmaj(br_gate[e])
        ubt[:, 2 + e, :] = _colmaj(br_up[e]) * SXW
    return _bf16(ws), _fp8(we, SWQ), gbt, ubt


def _run(inputs, trace=False):
    x = np.asarray(inputs["x"], np.float32)
    rweights = np.asarray(inputs["routing_weights"], np.float32)
    wsb, we8, gbt, ubt = _pack_shared(
        np.asarray(inputs["Ws_gate"], np.float32), inputs["bs_gate"],
        np.asarray(inputs["Ws_up"], np.float32), inputs["bs_up"],
        np.asarray(inputs["Ws_down"], np.float32), inputs["bs_down"],
        np.asarray(inputs["Wr_gate"], np.float32), inputs["br_gate"],
        np.asarray(inputs["Wr_up"], np.float32), inputs["br_up"],
        np.asarray(inputs["Wr_down"], np.float32), inputs["br_down"])
    bs_down = np.asarray(inputs["bs_down"], np.float32)
    br_down = np.asarray(inputs["br_down"], np.float32)
    # down-bias vector per batch: bs_down + sum_e rw[b,e]*br_down[e]
    cfull = bs_down[None, :] + rweights @ br_down       # [B, D]

    in_maps = []
    for i in range(NCORES):
        xT = x[BL * i:BL * (i + 1)].reshape(T, D).T     # [D, T]
        # pack x to [128, NK, T]: partition p, block k, token t = xT[k*128+p, t]
        xP = xT.reshape(NK, 128, T).transpose(1, 0, 2)
        rwtab = np.empty((128, NU, NTT), np.float32)
        rwtab[:, :2, :] = SDC
        cvtab = np.empty((128, ND, NTT), np.float32)
        for t in range(NTT):
            bg = BL * i + t // (K // TT)
            for e in range(E):
                rwtab[:, 2 + e, t] = rweights[bg, e] * (C / SXW)
            cvtab[:, :, t] = cfull[bg].reshape(ND, 128).T
        in_maps.append({"xTb": _bf16(xP), "xT8": _fp8(xP, SX),
                        "wsb": wsb, "we8": we8,
                        "gb": gbt, "ub": ubt,
                        "rw": np.ascontiguousarray(rwtab),
                        "cv": np.ascontiguousarray(cvtab)})

    fused = (not np.any(np.asarray(inputs["bs_up"], np.float32))
             and not np.any(np.asarray(inputs["br_up"], np.float32)))
    res = bass_utils.run_bass_kernel_spmd(_get_nc(fused), in_maps,
                                          core_ids=list(range(NCORES)),
                                          trace=trace)
    out = np.empty((B, K, D), np.float32)
    for i in range(NCORES):
        out[BL * i:BL * (i + 1)] = res.results[i]["outT"].T.reshape(BL, K, D)
    return out, res


def kernel(**inputs) -> np.ndarray:
    out, _ = _run(inputs, trace=False)
    return out
